# revision 2
# baseline (speedup 1.0000x reference)
"""Complex GRU cell on 8 Trainium2 NeuronCores (Bass/Tile).

Strategy
  - Data-parallel: batch 16384 -> 8 cores x 2048; 512x512 weights replicated.
  - Feature-major (transposed) layout on device: host pre-transposes x,h to
    [D, B_local] fp16 and pre-transposes the weights, so the kernel needs no
    on-device transposes and per-feature biases land on SBUF partitions
    (fused into ACT ops).
  - Complex matmul = 4 real fp16 matmuls accumulated into one fp32 PSUM bank;
    a pre-negated imaginary weight copy turns the subtraction into addition.
  - Per batch-chunk of 512: R wave (sigmoid gate), r*h on DVE, Z wave,
    C wave (x part + (r*h) part), polar-tanh via ACT Sqrt/Tanh + DVE
    reciprocal_approx_fast, final complex blend h + z*(h_tilde - h).
"""
import sys

for _p in ("/opt/trn_rl_repo",):
    if _p not in sys.path:
        sys.path.insert(0, _p)

import numpy as np
import concourse.bass as bass
import concourse.tile as tile
import concourse.mybir as mybir
from concourse.bass_utils import run_bass_kernel_spmd

F32, F16 = mybir.dt.float32, mybir.dt.float16
AF = mybir.ActivationFunctionType
ALU = mybir.AluOpType

RE, IM, IMN = 0, 1, 2  # weight variants: re, im, -im
GZ, GR, GH = 0, 1, 2   # gates (z, r, candidate)

N_CORES = 8
B_FULL, D, H = 16384, 512, 512
B_LOCAL = B_FULL // N_CORES
BCHUNK = 512

LAST_RUN_INFO = {}
_CACHE = {}


def _split_waits(nc, maxw=1):
    """walrus here allows 1 sync wait per instruction; hoist extras onto NoOps."""
    for fn in nc.m.functions:
        for bb in fn.blocks:
            out = []
            for inst in list(bb.instructions):
                si = inst.sync_info
                waits = list(si.on_wait) if si is not None else []
                if len(waits) > maxw:
                    extra, keep = waits[:-maxw], waits[-maxw:]
                    k = 0
                    while extra:
                        chunk, extra = extra[:maxw], extra[maxw:]
                        out.append(mybir.InstNoOp(
                            name=f"{inst.name}-wsplit{k}", engine=inst.engine,
                            ins=[], outs=[],
                            sync_info=mybir.SyncInfo(on_wait=chunk, on_update=[])))
                        k += 1
                    inst.sync_info = mybir.SyncInfo(on_wait=keep,
                                                    on_update=list(si.on_update))
                out.append(inst)
            bb.instructions[:] = out
    return nc


def _build(split_for_hw=True):
    NBC = B_LOCAL // BCHUNK
    nc = bass.Bass("TRN2", target_bir_lowering=False, debug=False)

    xr = nc.dram_tensor("xr", [4, 128, B_LOCAL], F16, kind="ExternalInput")
    xi = nc.dram_tensor("xi", [4, 128, B_LOCAL], F16, kind="ExternalInput")
    hr = nc.dram_tensor("hr", [4, 128, B_LOCAL], F16, kind="ExternalInput")
    hi = nc.dram_tensor("hi", [4, 128, B_LOCAL], F16, kind="ExternalInput")
    wx = nc.dram_tensor("wx", [3, 3, 4, 128, 512], F16, kind="ExternalInput")
    wh = nc.dram_tensor("wh", [3, 3, 4, 128, 512], F16, kind="ExternalInput")
    bias = nc.dram_tensor("bias", [3, 2, 4, 128], F32, kind="ExternalInput")
    outr = nc.dram_tensor("outr", [512, B_LOCAL], F32, kind="ExternalOutput")
    outi = nc.dram_tensor("outi", [512, B_LOCAL], F32, kind="ExternalOutput")

    with tile.TileContext(nc) as tc:
        with (
            tc.tile_pool(name="wpool", bufs=1) as wpool,
            tc.tile_pool(name="apool", bufs=2) as apool,
            tc.tile_pool(name="rhpool", bufs=2) as rhpool,
            tc.tile_pool(name="zpool", bufs=2) as zpool,
            tc.tile_pool(name="spool", bufs=2) as spool,
            tc.tile_pool(name="opool", bufs=3) as opool,
            tc.tile_pool(name="pspool", bufs=8, space="PSUM") as pspool,
        ):
            W = {}
            for which, src in (("x", wx), ("h", wh)):
                for g in range(3):
                    for v in range(3):
                        for dt in range(4):
                            t = wpool.tile([128, 512], F16, tag=f"w{which}{g}{v}{dt}")
                            nc.sync.dma_start(t[:], src[g, v, dt])
                            W[(which, g, v, dt)] = t
            BT = {}
            for g in range(3):
                for comp in range(2):
                    for t4 in range(4):
                        t = wpool.tile([128, 1], F32, tag=f"b{g}{comp}{t4}")
                        nc.sync.dma_start(
                            t[:], bias[g, comp, t4].rearrange("(p o) -> p o", o=1))
                        BT[(g, comp, t4)] = t

            def gate_mms(ps, g, comp, ax_pair, ah_pair, t4, first_start,
                         last_stop=True):
                # comp==0 (re): axr@Wre + axi@W(-im) + ahr@Whre + ahi@Wh(-im)
                # comp==1 (im): axr@Wim + axi@Wre   + ahr@Whim + ahi@Whre
                var_a, var_b = (RE, IMN) if comp == 0 else (IM, RE)
                terms = []
                if ax_pair is not None:
                    terms += [("x", var_a, ax_pair[0]), ("x", var_b, ax_pair[1])]
                if ah_pair is not None:
                    terms += [("h", var_a, ah_pair[0]), ("h", var_b, ah_pair[1])]
                n = len(terms) * 4
                i = 0
                for which, v, act in terms:
                    for dt in range(4):
                        nc.tensor.matmul(
                            ps[:],
                            W[(which, g, v, dt)][:, t4 * 128:(t4 + 1) * 128],
                            act[dt][:],
                            start=(first_start and i == 0),
                            stop=(last_stop and i == n - 1))
                        i += 1

            for bc in range(NBC):
                bsl = slice(bc * BCHUNK, (bc + 1) * BCHUNK)
                ax_re, ax_im, ah_re, ah_im = {}, {}, {}, {}
                for dt in range(4):
                    for nm, dram, dst in (("xr", xr, ax_re), ("xi", xi, ax_im),
                                          ("hr", hr, ah_re), ("hi", hi, ah_im)):
                        t = apool.tile([128, BCHUNK], F16, tag=f"a{nm}{dt}")
                        nc.sync.dma_start(t[:], dram[dt, :, bsl])
                        dst[dt] = t

                # R wave: r = cv_sigmoid(px1 + pr); rh = r*h
                r16, rh_re, rh_im = {}, {}, {}
                for t4 in range(4):
                    for comp in range(2):
                        ps = pspool.tile([128, BCHUNK], F32, tag="ps")
                        gate_mms(ps, GR, comp, (ax_re, ax_im), (ah_re, ah_im), t4, True)
                        rt = spool.tile([128, BCHUNK], F16, tag=f"r{comp}")
                        nc.scalar.activation(rt[:], ps[:], AF.Sigmoid,
                                             bias=BT[(GR, comp, t4)][:])
                        r16[(t4, comp)] = rt
                    rr, ri = r16[(t4, 0)], r16[(t4, 1)]
                    t1 = spool.tile([128, BCHUNK], F16, tag="t1")
                    t2 = spool.tile([128, BCHUNK], F16, tag="t2")
                    nc.vector.tensor_tensor(t1[:], rr[:], ah_re[t4][:], ALU.mult)
                    nc.vector.tensor_tensor(t2[:], ri[:], ah_im[t4][:], ALU.mult)
                    rhr = rhpool.tile([128, BCHUNK], F16, tag=f"rhr{t4}")
                    nc.vector.tensor_tensor(rhr[:], t1[:], t2[:], ALU.subtract)
                    t3 = spool.tile([128, BCHUNK], F16, tag="t1")
                    t4b = spool.tile([128, BCHUNK], F16, tag="t2")
                    nc.vector.tensor_tensor(t3[:], rr[:], ah_im[t4][:], ALU.mult)
                    nc.vector.tensor_tensor(t4b[:], ri[:], ah_re[t4][:], ALU.mult)
                    rhi = rhpool.tile([128, BCHUNK], F16, tag=f"rhi{t4}")
                    nc.vector.tensor_tensor(rhi[:], t3[:], t4b[:], ALU.add)
                    rh_re[t4], rh_im[t4] = rhr, rhi

                # Z wave: z = cv_sigmoid(px0 + pz)
                z16 = {}
                for t4 in range(4):
                    for comp in range(2):
                        ps = pspool.tile([128, BCHUNK], F32, tag="ps")
                        gate_mms(ps, GZ, comp, (ax_re, ax_im), (ah_re, ah_im), t4, True)
                        zt = zpool.tile([128, BCHUNK], F16, tag=f"z{t4}{comp}")
                        nc.scalar.activation(zt[:], ps[:], AF.Sigmoid,
                                             bias=BT[(GZ, comp, t4)][:])
                        z16[(t4, comp)] = zt

                # C wave: c = px2 + (r*h)@Wh2^T; h_tilde = polar_tanh(c + b)
                for t4 in range(4):
                    cps = {}
                    for comp in range(2):
                        ps = pspool.tile([128, BCHUNK], F32, tag="ps")
                        gate_mms(ps, GH, comp, (ax_re, ax_im), None, t4, True,
                                 last_stop=False)
                        gate_mms(ps, GH, comp, None, (rh_re, rh_im), t4, False)
                        cps[comp] = ps
                    bre, bim = BT[(GH, 0, t4)], BT[(GH, 1, t4)]
                    cbr = spool.tile([128, BCHUNK], F16, tag="cbr")
                    cbi = spool.tile([128, BCHUNK], F16, tag="cbi")
                    nc.scalar.activation(cbr[:], cps[0][:], AF.Identity, bias=bre[:])
                    nc.scalar.activation(cbi[:], cps[1][:], AF.Identity, bias=bim[:])
                    sre = spool.tile([128, BCHUNK], F32, tag="sre")
                    sim_ = spool.tile([128, BCHUNK], F32, tag="sim")
                    nc.scalar.activation(sre[:], cps[0][:], AF.Square, bias=bre[:])
                    nc.scalar.activation(sim_[:], cps[1][:], AF.Square, bias=bim[:])
                    m2 = spool.tile([128, BCHUNK], F32, tag="m2")
                    nc.vector.tensor_tensor(m2[:], sre[:], sim_[:], ALU.add)
                    mag = spool.tile([128, BCHUNK], F32, tag="mag")
                    nc.scalar.activation(mag[:], m2[:], AF.Sqrt)
                    th = spool.tile([128, BCHUNK], F32, tag="th")
                    nc.scalar.activation(th[:], mag[:], AF.Tanh)
                    # 1/mag = exp(-0.5*ln(m2)) on ACT (custom-DVE recip breaks
                    # this walrus build; DVE iterative reciprocal is ~8x slower)
                    lnm = spool.tile([128, BCHUNK], F32, tag="lnm")
                    nc.scalar.activation(lnm[:], m2[:], AF.Ln)
                    inv = spool.tile([128, BCHUNK], F32, tag="inv")
                    nc.scalar.activation(inv[:], lnm[:], AF.Exp, scale=-0.5)
                    tf = spool.tile([128, BCHUNK], F16, tag="tf")
                    nc.vector.tensor_tensor(tf[:], th[:], inv[:], ALU.mult)
                    htr = spool.tile([128, BCHUNK], F16, tag="htr")
                    hti = spool.tile([128, BCHUNK], F16, tag="hti")
                    nc.vector.tensor_tensor(htr[:], tf[:], cbr[:], ALU.mult)
                    nc.vector.tensor_tensor(hti[:], tf[:], cbi[:], ALU.mult)

                    # final: h_new = h + z*(h_tilde - h)
                    dre = spool.tile([128, BCHUNK], F16, tag="dre")
                    dim = spool.tile([128, BCHUNK], F16, tag="dim")
                    nc.vector.tensor_tensor(dre[:], htr[:], ah_re[t4][:], ALU.subtract)
                    nc.vector.tensor_tensor(dim[:], hti[:], ah_im[t4][:], ALU.subtract)
                    zr, zi = z16[(t4, 0)], z16[(t4, 1)]
                    u1 = spool.tile([128, BCHUNK], F16, tag="u1")
                    u2 = spool.tile([128, BCHUNK], F16, tag="u2")
                    nc.vector.tensor_tensor(u1[:], zr[:], dre[:], ALU.mult)
                    nc.vector.tensor_tensor(u2[:], zi[:], dim[:], ALU.mult)
                    ere = spool.tile([128, BCHUNK], F16, tag="ere")
                    nc.vector.tensor_tensor(ere[:], u1[:], u2[:], ALU.subtract)
                    u3 = spool.tile([128, BCHUNK], F16, tag="u1")
                    u4 = spool.tile([128, BCHUNK], F16, tag="u2")
                    nc.vector.tensor_tensor(u3[:], zr[:], dim[:], ALU.mult)
                    nc.vector.tensor_tensor(u4[:], zi[:], dre[:], ALU.mult)
                    eim = spool.tile([128, BCHUNK], F16, tag="eim")
                    nc.vector.tensor_tensor(eim[:], u3[:], u4[:], ALU.add)
                    orr = opool.tile([128, BCHUNK], F32, tag="or")
                    oii = opool.tile([128, BCHUNK], F32, tag="oi")
                    nc.vector.tensor_tensor(orr[:], ah_re[t4][:], ere[:], ALU.add)
                    nc.vector.tensor_tensor(oii[:], ah_im[t4][:], eim[:], ALU.add)
                    nc.sync.dma_start(outr[t4 * 128:(t4 + 1) * 128, bsl], orr[:])
                    nc.sync.dma_start(outi[t4 * 128:(t4 + 1) * 128, bsl], oii[:])

    if split_for_hw:
        _split_waits(nc)
    return nc


def _prep(inputs):
    x_re, x_im = inputs["x_re"], inputs["x_im"]
    h_re, h_im = inputs["h_re"], inputs["h_im"]

    def actT(a, sl):
        return np.ascontiguousarray(
            a[sl].T.reshape(4, 128, B_LOCAL).astype(np.float16))

    def wvar(Wre, Wim):
        out = np.empty((3, 3, 4, 128, 512), np.float16)
        for g in range(3):
            WreT, WimT = Wre[g].T, Wim[g].T
            out[g, RE] = WreT.reshape(4, 128, 512)
            out[g, IM] = WimT.reshape(4, 128, 512)
            out[g, IMN] = (-WimT).reshape(4, 128, 512)
        return out

    wxn = wvar(inputs["Wx_re"], inputs["Wx_im"])
    whn = wvar(inputs["Wh_re"], inputs["Wh_im"])
    bias = np.stack([inputs["bx_re"] + inputs["bh_re"],
                     inputs["bx_im"] + inputs["bh_im"]],
                    axis=1).reshape(3, 2, 4, 128).astype(np.float32)

    in_maps = []
    for c in range(N_CORES):
        sl = slice(c * B_LOCAL, (c + 1) * B_LOCAL)
        in_maps.append({
            "xr": actT(x_re, sl), "xi": actT(x_im, sl),
            "hr": actT(h_re, sl), "hi": actT(h_im, sl),
            "wx": wxn, "wh": whn, "bias": bias,
        })
    return in_maps


def kernel(**inputs):
    if "nc" not in _CACHE:
        nc = _build(split_for_hw=False)
        try:
            from concourse.timeline_sim import TimelineSim
            LAST_RUN_INFO["timeline_ns"] = int(TimelineSim(nc).simulate())
        except Exception:
            pass
        _CACHE["nc"] = _split_waits(nc)
    nc = _CACHE["nc"]

    in_maps = _prep(inputs)
    res = run_bass_kernel_spmd(nc, in_maps, list(range(N_CORES)))
    LAST_RUN_INFO["exec_time_ns"] = res.exec_time_ns

    out = np.empty((B_FULL, 512, 2), np.float32)
    for c, r in enumerate(res.results):
        sl = slice(c * B_LOCAL, (c + 1) * B_LOCAL)
        out[sl, :, 0] = r["outr"].T
        out[sl, :, 1] = r["outi"].T
    return out
